# revision 31
# baseline (speedup 1.0000x reference)
"""Trainium2 Bass kernel for nn_BinaryPathEncoder (v7 — fp16 tree, no gather).

Math: output row for position p is ones(256) pushed through a chain of
matrices P0/P1 chosen by the bits of p (LSB-first, topmost set bit dropped).
All distinct bit-paths form a complete binary tree with 2^17-1 nodes; level
k+1 of the tree is [P0 @ V_k, P1 @ V_k] so the whole tree costs ~17 GFLOP.
Every output row is then a lookup into the tree table.

Sharding: tree nodes are assigned to core p mod 8; children of a core's
node stay on that core, so each core's subtree is self-contained with zero
cross-core communication.  The host computes the tiny levels 0..11 (4095
rows, 3% of the table) and hands each core its 256 level-11 seed vectors;
the device builds levels 12..16 (15872 rows/core, ~97% of the FLOPs) in
column layout [dim, nodes]:

  child half (b,i) = sum_j P_b^T[128j:, 128i:]^T @ V[j]     (PSUM f32)

Numerics: everything on device is fp16 with a per-level 2^-4 rescale
(PSUM accumulates in f32; each drain multiplies by 0.0625).  A random
N(0,1) matvec grows vectors by ~sqrt(256)=16 per level, so scaled values
stay O(16) — far from fp16 limits — and the host undoes the exact
power-of-two scale.  fp16's 10-bit mantissa keeps the whole chain at
~1.8e-3 rel err (vs the 2e-2 envelope), and unlike fp32/fp32r the PE gets
fast weight loads (FWL), so matmuls pace at the streaming rate instead of
being LDWEIGHTS-bound.  Weights and seeds are pre-converted to fp16 on the
host, so the device consumes them straight from the input DMA.

No on-device gather: the full per-core table streams out (8.3 MB/core,
fp16) and the host does the final index lookup.  V levels are kept as
per-b-block tiles written b-major so the next level's whole-tile read
dependency resolves at the previous level's midpoint; drains alternate
DVE/ACT so neither engine trails the PE.
"""

import numpy as np

DIM = 256
NCORES = 8
DEV_LV0 = 12                       # first device-built level
L_MAX = 16                         # deepest tree level (positions < 2^17)
SEED_C = 1 << (DEV_LV0 - 4)        # level-(DEV_LV0-1) cols per core (256)
VMAX = 1 << (L_MAX - 4)            # widest chain level (level 15: 4096)
OUT_COLS = (1 << (L_MAX - 2)) - 2 * SEED_C   # cols for levels DEV_LV0..16
SEED_SCALE = 2.0 ** -(4 * (DEV_LV0 - 1))     # host pre-scale of seeds
DRAIN_SCALE = 2.0 ** -4            # per-level rescale applied at drain time
# stored level-k value = true * 2^-4k; host multiplies by 2^4k (exact)


# ---------------------------------------------------------------------------
# device program
# ---------------------------------------------------------------------------

def build_program():
    import concourse.tile as tile
    import concourse.mybir as mybir
    from concourse import bacc

    f32 = mybir.dt.float32
    f16 = mybir.dt.float16
    COPY = mybir.ActivationFunctionType.Copy

    nc = bacc.Bacc("TRN2", target_bir_lowering=False, debug=False,
                   num_devices=NCORES)

    primsT = nc.dram_tensor("primsT", [2, DIM, DIM], f16,
                            kind="ExternalInput").ap()
    seeds = nc.dram_tensor("seeds", [2, 128, SEED_C], f16,
                           kind="ExternalInput").ap()
    out = nc.dram_tensor("out", [2, 128, OUT_COLS], f16,
                         kind="ExternalOutput").ap()

    from contextlib import ExitStack
    with tile.TileContext(nc) as tc:
        with ExitStack() as ctx:
            cpool = ctx.enter_context(tc.tile_pool(name="consts", bufs=1))
            vpool = ctx.enter_context(tc.tile_pool(name="vbufs", bufs=2))
            hpool = ctx.enter_context(tc.tile_pool(name="emith", bufs=1))
            ppool = ctx.enter_context(tc.tile_pool(name="pc", bufs=4,
                                                   space="PSUM"))

            # ---- inputs (already fp16; no conversion needed) -------------
            vs = cpool.tile([128, 2 * SEED_C], f16, tag="vs", name="vs")
            nc.sync.dma_start(vs[:].rearrange("p (j m) -> p j m", j=2),
                              seeds.rearrange("j p m -> p j m"))
            pt = cpool.tile([128, 4 * DIM], f16, tag="pt", name="pt")
            prsrc = primsT.rearrange("b (j p) d -> p (b j) d", p=128)
            prdst = pt[:].rearrange("p (g d) -> p g d", g=4)
            nc.scalar.dma_start(prdst[:, :2], prsrc[:, :2])
            nc.sync.dma_start(prdst[:, 2:], prsrc[:, 2:])
            pT = [[pt[:, (2 * b + j) * DIM:(2 * b + j + 1) * DIM]
                   for j in range(2)] for b in range(2)]
            # V as two per-b-block tiles per half: a level's whole-tile read
            # dep then resolves at the previous level's midpoint.
            sh = SEED_C // 2
            V = [[vs[:, SEED_C * j + sh * pb:SEED_C * j + sh * (pb + 1)]
                  for j in range(2)] for pb in range(2)]

            # ---- PE warmup ------------------------------------------------
            # ~2.5us of throwaway matmuls while the input DMAs are in
            # flight: the HAM clock gate needs ~3.4us of sustained PE
            # activity to lift the PE from 1.2 to 2.4 GHz, so spend the
            # DMA wait warming it instead of running the first levels cold.
            wt = cpool.tile([128, 512], f16, tag="wt", name="wt")
            nc.vector.memset(wt[:], 0.0)
            for _ in range(6):
                wp = ppool.tile([128, 1024], f32, tag="PC",
                                name="wp")[:, :512]
                nc.tensor.matmul(wp, wt[:, :128], wt[:, :512],
                                 start=True, stop=True)

            # drains alternate DVE / ACT so neither trails the PE; both
            # apply the 2^-4 rescale during the PSUM->SBUF fp16 copy.
            flip = [0]

            def chain_drain(dst_ap, src_ap):
                if flip[0] % 2 == 0:
                    nc.vector.tensor_scalar_mul(dst_ap, src_ap, DRAIN_SCALE)
                else:
                    nc.scalar.activation(dst_ap, src_ap, COPY,
                                         scale=DRAIN_SCALE)
                flip[0] += 1

            # ---- levels DEV_LV0..L_MAX -----------------------------------
            c = SEED_C
            off = 0
            for k in range(DEV_LV0, L_MAX + 1):
                cc = 2 * c
                hb = c // 2                  # parent block width
                last = k == L_MAX
                if last:
                    emit = [hpool.tile([128, cc], f16, tag=f"H{j}",
                                       name=f"H{j}")
                            for j in range(2)]
                else:
                    newV = [[vpool.tile([128, VMAX // 2], f16,
                                        tag=f"V{b}{i}",
                                        name=f"V{k}_{b}{i}")[:, :c]
                             for i in range(2)] for b in range(2)]
                for b in range(2):
                    for pb in range(2):
                        for s0 in range(0, hb, 1024):
                            w = min(1024, hb - s0)
                            for i in range(2):
                                wslice = slice(128 * i, 128 * (i + 1))
                                ps = ppool.tile([128, 1024], f32, tag="PC",
                                                name="ps")[:, :w]
                                for q0 in range(0, w, 512):
                                    qw = min(512, w - q0)
                                    nc.tensor.matmul(
                                        ps[:, q0:q0 + qw],
                                        pT[b][0][:, wslice],
                                        V[pb][0][:, s0 + q0:s0 + q0 + qw],
                                        start=True, stop=False)
                                    nc.tensor.matmul(
                                        ps[:, q0:q0 + qw],
                                        pT[b][1][:, wslice],
                                        V[pb][1][:, s0 + q0:s0 + q0 + qw],
                                        start=False, stop=True)
                                d0 = pb * hb + s0
                                if last:
                                    chain_drain(
                                        emit[i][:, b * c + d0:
                                                b * c + d0 + w],
                                        ps[:, :w])
                                    if (d0 + w) % 2048 == 0:
                                        f0 = b * c + d0 + w - 2048
                                        nc.sync.dma_start(
                                            out[i, :, off + f0:
                                                off + f0 + 2048],
                                            emit[i][:, f0:f0 + 2048])
                                else:
                                    chain_drain(newV[b][i][:, d0:d0 + w],
                                                ps[:, :w])
                    if not last:
                        # block b complete -> stream it out on the ACT ring
                        # (the sync ring carries the level-16 flushes)
                        for i in range(2):
                            nc.scalar.dma_start(
                                out[i, :, off + b * c:off + (b + 1) * c],
                                newV[b][i][:, :c])
                off += cc
                if not last:
                    V = newV
                c = cc

    nc.compile()
    return nc


# ---------------------------------------------------------------------------
# host side
# ---------------------------------------------------------------------------

def _host_levels(primitives, identity):
    """Table rows for p < 2^DEV_LV0 (levels 0..DEV_LV0-1) in fp32."""
    T = np.zeros((1 << DEV_LV0, DIM), np.float32)
    T[0] = identity[0]
    T[1] = identity[0]
    for k in range(1, DEV_LV0):
        prev = T[1 << (k - 1):1 << k]
        half = 1 << (k - 1)
        T[1 << k:(1 << k) + half] = prev @ primitives[0].T
        T[(1 << k) + half:1 << (k + 1)] = prev @ primitives[1].T
    return T


_PROGRAM_CACHE = {}


def _run(unique, primitives, identity, **run_kwargs):
    from concourse.bass_utils import run_bass_kernel_spmd

    unique = np.asarray(unique)
    primitives = np.ascontiguousarray(np.asarray(primitives, np.float32))
    identity = np.ascontiguousarray(np.asarray(identity, np.float32))

    if "prog" not in _PROGRAM_CACHE:
        _PROGRAM_CACHE["prog"] = build_program()
    nc = _PROGRAM_CACHE["prog"]

    Th = _host_levels(primitives, identity)          # rows p < 2^DEV_LV0
    primsT = np.ascontiguousarray(
        primitives.transpose(0, 2, 1)).astype(np.float16)

    in_maps = []
    for core in range(NCORES):
        sc = Th[(1 << (DEV_LV0 - 1)) + core:1 << DEV_LV0:NCORES]  # [SEED_C, 256]
        seeds = np.ascontiguousarray(
            (sc.T * np.float32(SEED_SCALE)).reshape(2, 128, SEED_C)
        ).astype(np.float16)
        in_maps.append({"primsT": primsT, "seeds": seeds})

    res = run_bass_kernel_spmd(nc, in_maps, core_ids=list(range(NCORES)),
                               **run_kwargs)

    # assemble the full table (undoing the exact power-of-two scales),
    # then one bulk lookup
    Tfull = np.zeros((1 << (L_MAX + 1), DIM), np.float32)
    Tfull[:1 << DEV_LV0] = Th
    for core in range(NCORES):
        lo = np.asarray(res.results[core]["out"]).astype(np.float32)
        lo = lo.reshape(2 * 128, OUT_COLS)
        off = 0
        for k in range(DEV_LV0, L_MAX + 1):
            cc = 1 << (k - 3)
            unscale = np.float32(2.0 ** (4 * k))
            Tfull[(1 << k) + core:1 << (k + 1):NCORES] = \
                lo[:, off:off + cc].T * unscale
            off += cc
    outv = Tfull[unique.astype(np.int64)]
    return outv, res


def kernel(unique, primitives, identity):
    out, _ = _run(unique, primitives, identity)
    return out


if __name__ == "__main__":
    # tiny smoke run (full shapes) — prefer test.py for the real check
    rng = np.random.default_rng(0)
    u = rng.integers(0, 1 << 17, size=131072).astype(np.int32)
    prims = rng.standard_normal((2, DIM, DIM)).astype(np.float32)
    ones = np.ones((1, DIM), np.float32)
    out = kernel(u, prims, ones)
    print("kernel output", out.shape, out.dtype)


# revision 33
# speedup vs baseline: 1.0349x; 1.0349x over previous
"""Trainium2 Bass kernel for nn_BinaryPathEncoder (v7 — fp16 tree, no gather).

Math: output row for position p is ones(256) pushed through a chain of
matrices P0/P1 chosen by the bits of p (LSB-first, topmost set bit dropped).
All distinct bit-paths form a complete binary tree with 2^17-1 nodes; level
k+1 of the tree is [P0 @ V_k, P1 @ V_k] so the whole tree costs ~17 GFLOP.
Every output row is then a lookup into the tree table.

Sharding: tree nodes are assigned to core p mod 8; children of a core's
node stay on that core, so each core's subtree is self-contained with zero
cross-core communication.  The host computes the tiny levels 0..11 (4095
rows, 3% of the table) and hands each core its 256 level-11 seed vectors;
the device builds levels 12..16 (15872 rows/core, ~97% of the FLOPs) in
column layout [dim, nodes]:

  child half (b,i) = sum_j P_b^T[128j:, 128i:]^T @ V[j]     (PSUM f32)

Numerics: everything on device is fp16 with a per-level 2^-4 rescale
(PSUM accumulates in f32; each drain multiplies by 0.0625).  A random
N(0,1) matvec grows vectors by ~sqrt(256)=16 per level, so scaled values
stay O(16) — far from fp16 limits — and the host undoes the exact
power-of-two scale.  fp16's 10-bit mantissa keeps the whole chain at
~1.8e-3 rel err (vs the 2e-2 envelope), and unlike fp32/fp32r the PE gets
fast weight loads (FWL), so matmuls pace at the streaming rate instead of
being LDWEIGHTS-bound.  Weights and seeds are pre-converted to fp16 on the
host, so the device consumes them straight from the input DMA.

No on-device gather: the full per-core table streams out (8.3 MB/core,
fp16) and the host does the final index lookup.  V levels are kept as
per-b-block tiles written b-major so the next level's whole-tile read
dependency resolves at the previous level's midpoint; drains alternate
DVE/ACT so neither engine trails the PE.
"""

import numpy as np

DIM = 256
NCORES = 8
DEV_LV0 = 12                       # first device-built level
L_MAX = 16                         # deepest tree level (positions < 2^17)
SEED_C = 1 << (DEV_LV0 - 4)        # level-(DEV_LV0-1) cols per core (256)
VMAX = 1 << (L_MAX - 4)            # widest chain level (level 15: 4096)
OUT_COLS = (1 << (L_MAX - 2)) - 2 * SEED_C   # cols for levels DEV_LV0..16
SEED_SCALE = 2.0 ** -(4 * (DEV_LV0 - 1))     # host pre-scale of seeds
DRAIN_SCALE = 2.0 ** -4            # per-level rescale applied at drain time
# stored level-k value = true * 2^-4k; host multiplies by 2^4k (exact)


# ---------------------------------------------------------------------------
# device program
# ---------------------------------------------------------------------------

def build_program():
    import concourse.tile as tile
    import concourse.mybir as mybir
    from concourse import bacc

    f32 = mybir.dt.float32
    f16 = mybir.dt.float16
    COPY = mybir.ActivationFunctionType.Copy

    nc = bacc.Bacc("TRN2", target_bir_lowering=False, debug=False,
                   num_devices=NCORES)

    primsT = nc.dram_tensor("primsT", [2, DIM, DIM], f16,
                            kind="ExternalInput").ap()
    seeds = nc.dram_tensor("seeds", [2, 128, SEED_C], f16,
                           kind="ExternalInput").ap()
    out = nc.dram_tensor("out", [2, 128, OUT_COLS], f16,
                         kind="ExternalOutput").ap()

    from contextlib import ExitStack
    with tile.TileContext(nc) as tc:
        with ExitStack() as ctx:
            cpool = ctx.enter_context(tc.tile_pool(name="consts", bufs=1))
            vpool = ctx.enter_context(tc.tile_pool(name="vbufs", bufs=2))
            hpool = ctx.enter_context(tc.tile_pool(name="emith", bufs=1))
            ppool = ctx.enter_context(tc.tile_pool(name="pc", bufs=4,
                                                   space="PSUM"))

            # ---- inputs (already fp16; no conversion needed) -------------
            vs = cpool.tile([128, 2 * SEED_C], f16, tag="vs", name="vs")
            nc.sync.dma_start(vs[:].rearrange("p (j m) -> p j m", j=2),
                              seeds.rearrange("j p m -> p j m"))
            pt = cpool.tile([128, 4 * DIM], f16, tag="pt", name="pt")
            prsrc = primsT.rearrange("b (j p) d -> p (b j) d", p=128)
            prdst = pt[:].rearrange("p (g d) -> p g d", g=4)
            nc.scalar.dma_start(prdst[:, :2], prsrc[:, :2])
            nc.sync.dma_start(prdst[:, 2:], prsrc[:, 2:])
            pT = [[pt[:, (2 * b + j) * DIM:(2 * b + j + 1) * DIM]
                   for j in range(2)] for b in range(2)]
            # Wide V levels are kept as two per-b-block tiles per half so a
            # level's whole-tile read dep resolves at the previous level's
            # midpoint; narrow levels (<=1024 cols) live in single tiles
            # (Vfull) so their parents are read without the block split,
            # halving the matmul count of the small levels.
            sh = SEED_C // 2
            V = [[vs[:, SEED_C * j + sh * pb:SEED_C * j + sh * (pb + 1)]
                  for j in range(2)] for pb in range(2)]
            Vfull = [vs[:, SEED_C * j:SEED_C * (j + 1)] for j in range(2)]

            # ---- PE warmup ------------------------------------------------
            # ~2.5us of throwaway matmuls while the input DMAs are in
            # flight: the HAM clock gate needs ~3.4us of sustained PE
            # activity to lift the PE from 1.2 to 2.4 GHz, so spend the
            # DMA wait warming it instead of running the first levels cold.
            wt = cpool.tile([128, 512], f16, tag="wt", name="wt")
            nc.vector.memset(wt[:], 0.0)
            for _ in range(6):
                wp = ppool.tile([128, 1024], f32, tag="PC",
                                name="wp")[:, :512]
                nc.tensor.matmul(wp, wt[:, :128], wt[:, :512],
                                 start=True, stop=True)

            # drains alternate DVE / ACT so neither trails the PE; both
            # apply the 2^-4 rescale during the PSUM->SBUF fp16 copy.
            flip = [0]

            def chain_drain(dst_ap, src_ap):
                if flip[0] % 2 == 0:
                    nc.vector.tensor_scalar_mul(dst_ap, src_ap, DRAIN_SCALE)
                else:
                    nc.scalar.activation(dst_ap, src_ap, COPY,
                                         scale=DRAIN_SCALE)
                flip[0] += 1

            # ---- levels DEV_LV0..L_MAX -----------------------------------
            c = SEED_C
            off = 0
            for k in range(DEV_LV0, L_MAX + 1):
                cc = 2 * c
                hb = c // 2                  # parent block width
                last = k == L_MAX
                single = cc <= 1024          # narrow level -> single tiles
                newVfull = None
                if last:
                    emit = [hpool.tile([128, cc], f16, tag=f"H{j}",
                                       name=f"H{j}")
                            for j in range(2)]
                elif single:
                    newVfull = [vpool.tile([128, 1024], f16, tag=f"Vs{i}",
                                           name=f"Vs{k}_{i}")[:, :cc]
                                for i in range(2)]
                    newV = [[newVfull[i][:, b * c:(b + 1) * c]
                             for i in range(2)] for b in range(2)]
                else:
                    newV = [[vpool.tile([128, VMAX // 2], f16,
                                        tag=f"V{b}{i}",
                                        name=f"V{k}_{b}{i}")[:, :c]
                             for i in range(2)] for b in range(2)]
                # sub-phase list: merged scan when the parents live in a
                # single tile, per-parent-block scan otherwise
                subs = []
                if Vfull is not None:
                    for s0 in range(0, c, 1024):
                        w = min(1024, c - s0)
                        subs.append((Vfull[0][:, s0:s0 + w],
                                     Vfull[1][:, s0:s0 + w], s0, w))
                else:
                    for pb in range(2):
                        for s0 in range(0, hb, 1024):
                            w = min(1024, hb - s0)
                            subs.append((V[pb][0][:, s0:s0 + w],
                                         V[pb][1][:, s0:s0 + w],
                                         pb * hb + s0, w))
                for b in range(2):
                    for (src0, src1, d0, w) in subs:
                        for i in range(2):
                            wslice = slice(128 * i, 128 * (i + 1))
                            ps = ppool.tile([128, 1024], f32, tag="PC",
                                            name="ps")[:, :w]
                            for q0 in range(0, w, 512):
                                qw = min(512, w - q0)
                                nc.tensor.matmul(
                                    ps[:, q0:q0 + qw],
                                    pT[b][0][:, wslice],
                                    src0[:, q0:q0 + qw],
                                    start=True, stop=False)
                                nc.tensor.matmul(
                                    ps[:, q0:q0 + qw],
                                    pT[b][1][:, wslice],
                                    src1[:, q0:q0 + qw],
                                    start=False, stop=True)
                            if last:
                                chain_drain(
                                    emit[i][:, b * c + d0:b * c + d0 + w],
                                    ps[:, :w])
                                if b == 1 and d0 >= c - 2048:
                                    # closing block: flush per sub-phase,
                                    # rings split by half, to shorten the
                                    # end-of-kernel DMA tail
                                    eng = nc.scalar if i == 0 else nc.sync
                                    f0 = b * c + d0
                                    eng.dma_start(
                                        out[i, :, off + f0:off + f0 + w],
                                        emit[i][:, f0:f0 + w])
                                elif (d0 + w) % 2048 == 0:
                                    f0 = b * c + d0 + w - 2048
                                    nc.sync.dma_start(
                                        out[i, :, off + f0:
                                            off + f0 + 2048],
                                        emit[i][:, f0:f0 + 2048])
                            else:
                                chain_drain(newV[b][i][:, d0:d0 + w],
                                            ps[:, :w])
                    if not last:
                        # block b complete -> stream it out on the ACT ring
                        # (the sync ring carries the level-16 flushes)
                        for i in range(2):
                            nc.scalar.dma_start(
                                out[i, :, off + b * c:off + (b + 1) * c],
                                newV[b][i][:, :c])
                off += cc
                if not last:
                    V = newV
                    Vfull = newVfull
                c = cc

    nc.compile()
    return nc


# ---------------------------------------------------------------------------
# host side
# ---------------------------------------------------------------------------

def _host_levels(primitives, identity):
    """Table rows for p < 2^DEV_LV0 (levels 0..DEV_LV0-1) in fp32."""
    T = np.zeros((1 << DEV_LV0, DIM), np.float32)
    T[0] = identity[0]
    T[1] = identity[0]
    for k in range(1, DEV_LV0):
        prev = T[1 << (k - 1):1 << k]
        half = 1 << (k - 1)
        T[1 << k:(1 << k) + half] = prev @ primitives[0].T
        T[(1 << k) + half:1 << (k + 1)] = prev @ primitives[1].T
    return T


_PROGRAM_CACHE = {}


def _run(unique, primitives, identity, **run_kwargs):
    from concourse.bass_utils import run_bass_kernel_spmd

    unique = np.asarray(unique)
    primitives = np.ascontiguousarray(np.asarray(primitives, np.float32))
    identity = np.ascontiguousarray(np.asarray(identity, np.float32))

    if "prog" not in _PROGRAM_CACHE:
        _PROGRAM_CACHE["prog"] = build_program()
    nc = _PROGRAM_CACHE["prog"]

    Th = _host_levels(primitives, identity)          # rows p < 2^DEV_LV0
    primsT = np.ascontiguousarray(
        primitives.transpose(0, 2, 1)).astype(np.float16)

    in_maps = []
    for core in range(NCORES):
        sc = Th[(1 << (DEV_LV0 - 1)) + core:1 << DEV_LV0:NCORES]  # [SEED_C, 256]
        seeds = np.ascontiguousarray(
            (sc.T * np.float32(SEED_SCALE)).reshape(2, 128, SEED_C)
        ).astype(np.float16)
        in_maps.append({"primsT": primsT, "seeds": seeds})

    res = run_bass_kernel_spmd(nc, in_maps, core_ids=list(range(NCORES)),
                               **run_kwargs)

    # assemble the full table (undoing the exact power-of-two scales),
    # then one bulk lookup
    Tfull = np.zeros((1 << (L_MAX + 1), DIM), np.float32)
    Tfull[:1 << DEV_LV0] = Th
    for core in range(NCORES):
        lo = np.asarray(res.results[core]["out"]).astype(np.float32)
        lo = lo.reshape(2 * 128, OUT_COLS)
        off = 0
        for k in range(DEV_LV0, L_MAX + 1):
            cc = 1 << (k - 3)
            unscale = np.float32(2.0 ** (4 * k))
            Tfull[(1 << k) + core:1 << (k + 1):NCORES] = \
                lo[:, off:off + cc].T * unscale
            off += cc
    outv = Tfull[unique.astype(np.int64)]
    return outv, res


def kernel(unique, primitives, identity):
    out, _ = _run(unique, primitives, identity)
    return out


if __name__ == "__main__":
    # tiny smoke run (full shapes) — prefer test.py for the real check
    rng = np.random.default_rng(0)
    u = rng.integers(0, 1 << 17, size=131072).astype(np.int32)
    prims = rng.standard_normal((2, DIM, DIM)).astype(np.float32)
    ones = np.ones((1, DIM), np.float32)
    out = kernel(u, prims, ones)
    print("kernel output", out.shape, out.dtype)


# revision 35
# speedup vs baseline: 1.1175x; 1.0798x over previous
"""Trainium2 Bass kernel for nn_BinaryPathEncoder (v7 — fp16 tree, no gather).

Math: output row for position p is ones(256) pushed through a chain of
matrices P0/P1 chosen by the bits of p (LSB-first, topmost set bit dropped).
All distinct bit-paths form a complete binary tree with 2^17-1 nodes; level
k+1 of the tree is [P0 @ V_k, P1 @ V_k] so the whole tree costs ~17 GFLOP.
Every output row is then a lookup into the tree table.

Sharding: tree nodes are assigned to core p mod 8; children of a core's
node stay on that core, so each core's subtree is self-contained with zero
cross-core communication.  The host computes the tiny levels 0..11 (4095
rows, 3% of the table) and hands each core its 256 level-11 seed vectors;
the device builds levels 12..16 (15872 rows/core, ~97% of the FLOPs) in
column layout [dim, nodes]:

  child half (b,i) = sum_j P_b^T[128j:, 128i:]^T @ V[j]     (PSUM f32)

Numerics: everything on device is fp16 with a per-level 2^-4 rescale
(PSUM accumulates in f32; each drain multiplies by 0.0625).  A random
N(0,1) matvec grows vectors by ~sqrt(256)=16 per level, so scaled values
stay O(16) — far from fp16 limits — and the host undoes the exact
power-of-two scale.  fp16's 10-bit mantissa keeps the whole chain at
~1.8e-3 rel err (vs the 2e-2 envelope), and unlike fp32/fp32r the PE gets
fast weight loads (FWL), so matmuls pace at the streaming rate instead of
being LDWEIGHTS-bound.  Weights and seeds are pre-converted to fp16 on the
host, so the device consumes them straight from the input DMA.

No on-device gather: the full per-core table streams out (8.3 MB/core,
fp16) and the host does the final index lookup.  V levels are kept as
per-b-block tiles written b-major so the next level's whole-tile read
dependency resolves at the previous level's midpoint; drains alternate
DVE/ACT so neither engine trails the PE.
"""

import numpy as np

DIM = 256
NCORES = 8
DEV_LV0 = 12                       # first device-built level
L_MAX = 16                         # deepest tree level (positions < 2^17)
SEED_C = 1 << (DEV_LV0 - 4)        # level-(DEV_LV0-1) cols per core (256)
VMAX = 1 << (L_MAX - 4)            # widest chain level (level 15: 4096)
OUT_COLS = (1 << (L_MAX - 2)) - 2 * SEED_C   # cols for levels DEV_LV0..16
SEED_SCALE = 2.0 ** -(4 * (DEV_LV0 - 1))     # host pre-scale of seeds
DRAIN_SCALE = 2.0 ** -4            # per-level rescale applied at drain time
# stored level-k value = true * 2^-4k; host multiplies by 2^4k (exact)


# ---------------------------------------------------------------------------
# device program
# ---------------------------------------------------------------------------

def build_program():
    import concourse.tile as tile
    import concourse.mybir as mybir
    from concourse import bacc

    f32 = mybir.dt.float32
    f16 = mybir.dt.float16
    COPY = mybir.ActivationFunctionType.Copy

    nc = bacc.Bacc("TRN2", target_bir_lowering=False, debug=False,
                   num_devices=NCORES)

    primsT = nc.dram_tensor("primsT", [2, DIM, DIM], f16,
                            kind="ExternalInput").ap()
    seeds = nc.dram_tensor("seeds", [2, 128, SEED_C], f16,
                           kind="ExternalInput").ap()
    out = nc.dram_tensor("out", [2, 128, OUT_COLS], f16,
                         kind="ExternalOutput").ap()

    from contextlib import ExitStack
    with tile.TileContext(nc) as tc:
        with ExitStack() as ctx:
            cpool = ctx.enter_context(tc.tile_pool(name="consts", bufs=1))
            vpool = ctx.enter_context(tc.tile_pool(name="vbufs", bufs=2))
            hpool = ctx.enter_context(tc.tile_pool(name="emith", bufs=1))
            ppool = ctx.enter_context(tc.tile_pool(name="pc", bufs=4,
                                                   space="PSUM"))

            # ---- inputs (already fp16; no conversion needed) -------------
            vs = cpool.tile([128, 2 * SEED_C], f16, tag="vs", name="vs")
            nc.sync.dma_start(vs[:].rearrange("p (j m) -> p j m", j=2),
                              seeds.rearrange("j p m -> p j m"))
            pt = cpool.tile([128, 4 * DIM], f16, tag="pt", name="pt")
            prsrc = primsT.rearrange("b (j p) d -> p (b j) d", p=128)
            prdst = pt[:].rearrange("p (g d) -> p g d", g=4)
            nc.scalar.dma_start(prdst[:, :2], prsrc[:, :2])
            nc.sync.dma_start(prdst[:, 2:], prsrc[:, 2:])
            pT = [[pt[:, (2 * b + j) * DIM:(2 * b + j + 1) * DIM]
                   for j in range(2)] for b in range(2)]
            # Wide V levels are kept as two per-b-block tiles per half so a
            # level's whole-tile read dep resolves at the previous level's
            # midpoint; narrow levels (<=1024 cols) live in single tiles
            # (Vfull) so their parents are read without the block split,
            # halving the matmul count of the small levels.
            sh = SEED_C // 2
            V = [[vs[:, SEED_C * j + sh * pb:SEED_C * j + sh * (pb + 1)]
                  for j in range(2)] for pb in range(2)]
            Vfull = [vs[:, SEED_C * j:SEED_C * (j + 1)] for j in range(2)]

            # ---- PE warmup ------------------------------------------------
            # ~2.5us of throwaway matmuls while the input DMAs are in
            # flight: the HAM clock gate needs ~3.4us of sustained PE
            # activity to lift the PE from 1.2 to 2.4 GHz, so spend the
            # DMA wait warming it instead of running the first levels cold.
            wt = cpool.tile([128, 512], f16, tag="wt", name="wt")
            nc.vector.memset(wt[:], 0.0)
            for _ in range(6):
                wp = ppool.tile([128, 1024], f32, tag="PC",
                                name="wp")[:, :512]
                nc.tensor.matmul(wp, wt[:, :128], wt[:, :512],
                                 start=True, stop=True)

            # drains alternate DVE / ACT so neither trails the PE; both
            # apply the 2^-4 rescale during the PSUM->SBUF fp16 copy.
            flip = [0]

            def chain_drain(dst_ap, src_ap):
                if flip[0] % 2 == 0:
                    nc.vector.tensor_scalar_mul(dst_ap, src_ap, DRAIN_SCALE)
                else:
                    nc.scalar.activation(dst_ap, src_ap, COPY,
                                         scale=DRAIN_SCALE)
                flip[0] += 1

            # ---- levels DEV_LV0..L_MAX -----------------------------------
            c = SEED_C
            off = 0
            for k in range(DEV_LV0, L_MAX + 1):
                cc = 2 * c
                hb = c // 2                  # parent block width
                last = k == L_MAX
                single = cc <= 1024          # narrow level -> single tiles
                newVfull = None
                if last:
                    emit = [hpool.tile([128, cc], f16, tag=f"H{j}",
                                       name=f"H{j}")
                            for j in range(2)]
                elif single:
                    newVfull = [vpool.tile([128, 1024], f16, tag=f"Vs{i}",
                                           name=f"Vs{k}_{i}")[:, :cc]
                                for i in range(2)]
                    newV = [[newVfull[i][:, b * c:(b + 1) * c]
                             for i in range(2)] for b in range(2)]
                else:
                    newV = [[vpool.tile([128, VMAX // 2], f16,
                                        tag=f"V{b}{i}",
                                        name=f"V{k}_{b}{i}")[:, :c]
                             for i in range(2)] for b in range(2)]
                # sub-phase list: merged scan when the parents live in a
                # single tile, per-parent-block scan otherwise
                subs = []
                if Vfull is not None:
                    for s0 in range(0, c, 1024):
                        w = min(1024, c - s0)
                        subs.append((Vfull[0][:, s0:s0 + w],
                                     Vfull[1][:, s0:s0 + w], s0, w))
                else:
                    for pb in range(2):
                        for s0 in range(0, hb, 1024):
                            w = min(1024, hb - s0)
                            subs.append((V[pb][0][:, s0:s0 + w],
                                         V[pb][1][:, s0:s0 + w],
                                         pb * hb + s0, w))
                for b in range(2):
                    for (src0, src1, d0, w) in subs:
                        for i in range(2):
                            wslice = slice(128 * i, 128 * (i + 1))
                            ps = ppool.tile([128, 1024], f32, tag="PC",
                                            name="ps")[:, :w]
                            for q0 in range(0, w, 512):
                                qw = min(512, w - q0)
                                nc.tensor.matmul(
                                    ps[:, q0:q0 + qw],
                                    pT[b][0][:, wslice],
                                    src0[:, q0:q0 + qw],
                                    start=True, stop=False)
                                nc.tensor.matmul(
                                    ps[:, q0:q0 + qw],
                                    pT[b][1][:, wslice],
                                    src1[:, q0:q0 + qw],
                                    start=False, stop=True)
                            if last:
                                chain_drain(
                                    emit[i][:, b * c + d0:b * c + d0 + w],
                                    ps[:, :w])
                                if b == 1 and d0 >= c - 2048:
                                    # closing block: flush per sub-phase to
                                    # shorten the end-of-kernel DMA tail
                                    f0 = b * c + d0
                                    nc.sync.dma_start(
                                        out[i, :, off + f0:off + f0 + w],
                                        emit[i][:, f0:f0 + w])
                                elif (d0 + w) % 2048 == 0:
                                    f0 = b * c + d0 + w - 2048
                                    nc.sync.dma_start(
                                        out[i, :, off + f0:
                                            off + f0 + 2048],
                                        emit[i][:, f0:f0 + 2048])
                            else:
                                chain_drain(newV[b][i][:, d0:d0 + w],
                                            ps[:, :w])
                    if not last:
                        # block b complete -> stream it out.  All output
                        # DMAs go on the sync ring: a dma_start issued from
                        # the ACT queue would head-of-line block the ACT
                        # drains behind its source-tile semaphore, and the
                        # drains are in the PE's PSUM-recycling path.
                        for i in range(2):
                            nc.sync.dma_start(
                                out[i, :, off + b * c:off + (b + 1) * c],
                                newV[b][i][:, :c])
                off += cc
                if not last:
                    V = newV
                    Vfull = newVfull
                c = cc

    nc.compile()
    return nc


# ---------------------------------------------------------------------------
# host side
# ---------------------------------------------------------------------------

def _host_levels(primitives, identity):
    """Table rows for p < 2^DEV_LV0 (levels 0..DEV_LV0-1) in fp32."""
    T = np.zeros((1 << DEV_LV0, DIM), np.float32)
    T[0] = identity[0]
    T[1] = identity[0]
    for k in range(1, DEV_LV0):
        prev = T[1 << (k - 1):1 << k]
        half = 1 << (k - 1)
        T[1 << k:(1 << k) + half] = prev @ primitives[0].T
        T[(1 << k) + half:1 << (k + 1)] = prev @ primitives[1].T
    return T


_PROGRAM_CACHE = {}


def _run(unique, primitives, identity, **run_kwargs):
    from concourse.bass_utils import run_bass_kernel_spmd

    unique = np.asarray(unique)
    primitives = np.ascontiguousarray(np.asarray(primitives, np.float32))
    identity = np.ascontiguousarray(np.asarray(identity, np.float32))

    if "prog" not in _PROGRAM_CACHE:
        _PROGRAM_CACHE["prog"] = build_program()
    nc = _PROGRAM_CACHE["prog"]

    Th = _host_levels(primitives, identity)          # rows p < 2^DEV_LV0
    primsT = np.ascontiguousarray(
        primitives.transpose(0, 2, 1)).astype(np.float16)

    in_maps = []
    for core in range(NCORES):
        sc = Th[(1 << (DEV_LV0 - 1)) + core:1 << DEV_LV0:NCORES]  # [SEED_C, 256]
        seeds = np.ascontiguousarray(
            (sc.T * np.float32(SEED_SCALE)).reshape(2, 128, SEED_C)
        ).astype(np.float16)
        in_maps.append({"primsT": primsT, "seeds": seeds})

    res = run_bass_kernel_spmd(nc, in_maps, core_ids=list(range(NCORES)),
                               **run_kwargs)

    # assemble the full table (undoing the exact power-of-two scales),
    # then one bulk lookup
    Tfull = np.zeros((1 << (L_MAX + 1), DIM), np.float32)
    Tfull[:1 << DEV_LV0] = Th
    for core in range(NCORES):
        lo = np.asarray(res.results[core]["out"]).astype(np.float32)
        lo = lo.reshape(2 * 128, OUT_COLS)
        off = 0
        for k in range(DEV_LV0, L_MAX + 1):
            cc = 1 << (k - 3)
            unscale = np.float32(2.0 ** (4 * k))
            Tfull[(1 << k) + core:1 << (k + 1):NCORES] = \
                lo[:, off:off + cc].T * unscale
            off += cc
    outv = Tfull[unique.astype(np.int64)]
    return outv, res


def kernel(unique, primitives, identity):
    out, _ = _run(unique, primitives, identity)
    return out


if __name__ == "__main__":
    # tiny smoke run (full shapes) — prefer test.py for the real check
    rng = np.random.default_rng(0)
    u = rng.integers(0, 1 << 17, size=131072).astype(np.int32)
    prims = rng.standard_normal((2, DIM, DIM)).astype(np.float32)
    ones = np.ones((1, DIM), np.float32)
    out = kernel(u, prims, ones)
    print("kernel output", out.shape, out.dtype)


# revision 37
# speedup vs baseline: 1.1318x; 1.0127x over previous
"""Trainium2 Bass kernel for nn_BinaryPathEncoder (v7 — fp16 tree, no gather).

Math: output row for position p is ones(256) pushed through a chain of
matrices P0/P1 chosen by the bits of p (LSB-first, topmost set bit dropped).
All distinct bit-paths form a complete binary tree with 2^17-1 nodes; level
k+1 of the tree is [P0 @ V_k, P1 @ V_k] so the whole tree costs ~17 GFLOP.
Every output row is then a lookup into the tree table.

Sharding: tree nodes are assigned to core p mod 8; children of a core's
node stay on that core, so each core's subtree is self-contained with zero
cross-core communication.  The host computes the tiny levels 0..11 (4095
rows, 3% of the table) and hands each core its 256 level-11 seed vectors;
the device builds levels 12..16 (15872 rows/core, ~97% of the FLOPs) in
column layout [dim, nodes]:

  child half (b,i) = sum_j P_b^T[128j:, 128i:]^T @ V[j]     (PSUM f32)

Numerics: everything on device is fp16 with a per-level 2^-4 rescale
(PSUM accumulates in f32; each drain multiplies by 0.0625).  A random
N(0,1) matvec grows vectors by ~sqrt(256)=16 per level, so scaled values
stay O(16) — far from fp16 limits — and the host undoes the exact
power-of-two scale.  fp16's 10-bit mantissa keeps the whole chain at
~1.8e-3 rel err (vs the 2e-2 envelope), and unlike fp32/fp32r the PE gets
fast weight loads (FWL), so matmuls pace at the streaming rate instead of
being LDWEIGHTS-bound.  Weights and seeds are pre-converted to fp16 on the
host, so the device consumes them straight from the input DMA.

No on-device gather: the full per-core table streams out (8.3 MB/core,
fp16) and the host does the final index lookup.  V levels are kept as
per-b-block tiles written b-major so the next level's whole-tile read
dependency resolves at the previous level's midpoint; drains alternate
DVE/ACT so neither engine trails the PE.
"""

import numpy as np

DIM = 256
NCORES = 8
DEV_LV0 = 12                       # first device-built level
L_MAX = 16                         # deepest tree level (positions < 2^17)
SEED_C = 1 << (DEV_LV0 - 4)        # level-(DEV_LV0-1) cols per core (256)
VMAX = 1 << (L_MAX - 4)            # widest chain level (level 15: 4096)
OUT_COLS = (1 << (L_MAX - 2)) - 2 * SEED_C   # cols for levels DEV_LV0..16
SEED_SCALE = 2.0 ** -(4 * (DEV_LV0 - 1))     # host pre-scale of seeds
DRAIN_SCALE = 2.0 ** -4            # per-level rescale applied at drain time
# stored level-k value = true * 2^-4k; host multiplies by 2^4k (exact)


# ---------------------------------------------------------------------------
# device program
# ---------------------------------------------------------------------------

def build_program():
    import concourse.tile as tile
    import concourse.mybir as mybir
    from concourse import bacc

    f32 = mybir.dt.float32
    f16 = mybir.dt.float16
    COPY = mybir.ActivationFunctionType.Copy

    nc = bacc.Bacc("TRN2", target_bir_lowering=False, debug=False,
                   num_devices=NCORES)

    primsT = nc.dram_tensor("primsT", [2, DIM, DIM], f16,
                            kind="ExternalInput").ap()
    seeds = nc.dram_tensor("seeds", [2, 128, SEED_C], f16,
                           kind="ExternalInput").ap()
    out = nc.dram_tensor("out", [2, 128, OUT_COLS], f16,
                         kind="ExternalOutput").ap()

    from contextlib import ExitStack
    with tile.TileContext(nc) as tc:
        with ExitStack() as ctx:
            cpool = ctx.enter_context(tc.tile_pool(name="consts", bufs=1))
            vpool = ctx.enter_context(tc.tile_pool(name="vbufs", bufs=2))
            hpool = ctx.enter_context(tc.tile_pool(name="emith", bufs=1))
            ppool = ctx.enter_context(tc.tile_pool(name="pc", bufs=4,
                                                   space="PSUM"))

            # ---- inputs (already fp16; no conversion needed) -------------
            vs = cpool.tile([128, 2 * SEED_C], f16, tag="vs", name="vs")
            nc.sync.dma_start(vs[:].rearrange("p (j m) -> p j m", j=2),
                              seeds.rearrange("j p m -> p j m"))
            pt = cpool.tile([128, 4 * DIM], f16, tag="pt", name="pt")
            prsrc = primsT.rearrange("b (j p) d -> p (b j) d", p=128)
            prdst = pt[:].rearrange("p (g d) -> p g d", g=4)
            nc.scalar.dma_start(prdst[:, :2], prsrc[:, :2])
            nc.sync.dma_start(prdst[:, 2:], prsrc[:, 2:])
            pT = [[pt[:, (2 * b + j) * DIM:(2 * b + j + 1) * DIM]
                   for j in range(2)] for b in range(2)]
            # Wide V levels are kept as two per-b-block tiles per half so a
            # level's whole-tile read dep resolves at the previous level's
            # midpoint; narrow levels (<=1024 cols) live in single tiles
            # (Vfull) so their parents are read without the block split,
            # halving the matmul count of the small levels.
            sh = SEED_C // 2
            V = [[vs[:, SEED_C * j + sh * pb:SEED_C * j + sh * (pb + 1)]
                  for j in range(2)] for pb in range(2)]
            Vfull = [vs[:, SEED_C * j:SEED_C * (j + 1)] for j in range(2)]

            # ---- PE warmup ------------------------------------------------
            # ~2.5us of throwaway matmuls while the input DMAs are in
            # flight: the HAM clock gate needs ~3.4us of sustained PE
            # activity to lift the PE from 1.2 to 2.4 GHz, so spend the
            # DMA wait warming it instead of running the first levels cold.
            wt = cpool.tile([128, 512], f16, tag="wt", name="wt")
            nc.vector.memset(wt[:], 0.0)
            for _ in range(5):
                wp = ppool.tile([128, 1024], f32, tag="PC",
                                name="wp")[:, :512]
                nc.tensor.matmul(wp, wt[:, :128], wt[:, :512],
                                 start=True, stop=True)

            # drains alternate DVE / ACT so neither trails the PE; both
            # apply the 2^-4 rescale during the PSUM->SBUF fp16 copy.
            flip = [0]

            def chain_drain(dst_ap, src_ap):
                if flip[0] % 2 == 0:
                    nc.vector.tensor_scalar_mul(dst_ap, src_ap, DRAIN_SCALE)
                else:
                    nc.scalar.activation(dst_ap, src_ap, COPY,
                                         scale=DRAIN_SCALE)
                flip[0] += 1

            # ---- levels DEV_LV0..L_MAX -----------------------------------
            c = SEED_C
            off = 0
            for k in range(DEV_LV0, L_MAX + 1):
                cc = 2 * c
                hb = c // 2                  # parent block width
                last = k == L_MAX
                single = cc <= 1024          # narrow level -> single tiles
                newVfull = None
                if last:
                    emit = [hpool.tile([128, cc], f16, tag=f"H{j}",
                                       name=f"H{j}")
                            for j in range(2)]
                elif single:
                    newVfull = [vpool.tile([128, 1024], f16, tag=f"Vs{i}",
                                           name=f"Vs{k}_{i}")[:, :cc]
                                for i in range(2)]
                    newV = [[newVfull[i][:, b * c:(b + 1) * c]
                             for i in range(2)] for b in range(2)]
                else:
                    newV = [[vpool.tile([128, VMAX // 2], f16,
                                        tag=f"V{b}{i}",
                                        name=f"V{k}_{b}{i}")[:, :c]
                             for i in range(2)] for b in range(2)]
                # sub-phase list: merged scan when the parents live in a
                # single tile, per-parent-block scan otherwise
                subs = []
                if Vfull is not None:
                    for s0 in range(0, c, 1024):
                        w = min(1024, c - s0)
                        subs.append((Vfull[0][:, s0:s0 + w],
                                     Vfull[1][:, s0:s0 + w], s0, w))
                else:
                    for pb in range(2):
                        for s0 in range(0, hb, 1024):
                            w = min(1024, hb - s0)
                            subs.append((V[pb][0][:, s0:s0 + w],
                                         V[pb][1][:, s0:s0 + w],
                                         pb * hb + s0, w))
                for b in range(2):
                    for (src0, src1, d0, w) in subs:
                        for i in range(2):
                            wslice = slice(128 * i, 128 * (i + 1))
                            ps = ppool.tile([128, 1024], f32, tag="PC",
                                            name="ps")[:, :w]
                            for q0 in range(0, w, 512):
                                qw = min(512, w - q0)
                                nc.tensor.matmul(
                                    ps[:, q0:q0 + qw],
                                    pT[b][0][:, wslice],
                                    src0[:, q0:q0 + qw],
                                    start=True, stop=False)
                                nc.tensor.matmul(
                                    ps[:, q0:q0 + qw],
                                    pT[b][1][:, wslice],
                                    src1[:, q0:q0 + qw],
                                    start=False, stop=True)
                            if last:
                                chain_drain(
                                    emit[i][:, b * c + d0:b * c + d0 + w],
                                    ps[:, :w])
                                if b == 1 and d0 >= c - 2048:
                                    # closing block: flush per sub-phase,
                                    # halves on separate rings so the two
                                    # final receipts overlap (ACT's queue is
                                    # empty by now, no head-of-line risk)
                                    eng = nc.scalar if i == 0 else nc.sync
                                    f0 = b * c + d0
                                    eng.dma_start(
                                        out[i, :, off + f0:off + f0 + w],
                                        emit[i][:, f0:f0 + w])
                                elif (d0 + w) % 2048 == 0:
                                    f0 = b * c + d0 + w - 2048
                                    nc.sync.dma_start(
                                        out[i, :, off + f0:
                                            off + f0 + 2048],
                                        emit[i][:, f0:f0 + 2048])
                            else:
                                chain_drain(newV[b][i][:, d0:d0 + w],
                                            ps[:, :w])
                    if not last:
                        # block b complete -> stream it out.  All output
                        # DMAs go on the sync ring: a dma_start issued from
                        # the ACT queue would head-of-line block the ACT
                        # drains behind its source-tile semaphore, and the
                        # drains are in the PE's PSUM-recycling path.
                        for i in range(2):
                            nc.sync.dma_start(
                                out[i, :, off + b * c:off + (b + 1) * c],
                                newV[b][i][:, :c])
                off += cc
                if not last:
                    V = newV
                    Vfull = newVfull
                c = cc

    nc.compile()
    return nc


# ---------------------------------------------------------------------------
# host side
# ---------------------------------------------------------------------------

def _host_levels(primitives, identity):
    """Table rows for p < 2^DEV_LV0 (levels 0..DEV_LV0-1) in fp32."""
    T = np.zeros((1 << DEV_LV0, DIM), np.float32)
    T[0] = identity[0]
    T[1] = identity[0]
    for k in range(1, DEV_LV0):
        prev = T[1 << (k - 1):1 << k]
        half = 1 << (k - 1)
        T[1 << k:(1 << k) + half] = prev @ primitives[0].T
        T[(1 << k) + half:1 << (k + 1)] = prev @ primitives[1].T
    return T


_PROGRAM_CACHE = {}


def _run(unique, primitives, identity, **run_kwargs):
    from concourse.bass_utils import run_bass_kernel_spmd

    unique = np.asarray(unique)
    primitives = np.ascontiguousarray(np.asarray(primitives, np.float32))
    identity = np.ascontiguousarray(np.asarray(identity, np.float32))

    if "prog" not in _PROGRAM_CACHE:
        _PROGRAM_CACHE["prog"] = build_program()
    nc = _PROGRAM_CACHE["prog"]

    Th = _host_levels(primitives, identity)          # rows p < 2^DEV_LV0
    primsT = np.ascontiguousarray(
        primitives.transpose(0, 2, 1)).astype(np.float16)

    in_maps = []
    for core in range(NCORES):
        sc = Th[(1 << (DEV_LV0 - 1)) + core:1 << DEV_LV0:NCORES]  # [SEED_C, 256]
        seeds = np.ascontiguousarray(
            (sc.T * np.float32(SEED_SCALE)).reshape(2, 128, SEED_C)
        ).astype(np.float16)
        in_maps.append({"primsT": primsT, "seeds": seeds})

    res = run_bass_kernel_spmd(nc, in_maps, core_ids=list(range(NCORES)),
                               **run_kwargs)

    # assemble the full table (undoing the exact power-of-two scales),
    # then one bulk lookup
    Tfull = np.zeros((1 << (L_MAX + 1), DIM), np.float32)
    Tfull[:1 << DEV_LV0] = Th
    for core in range(NCORES):
        lo = np.asarray(res.results[core]["out"]).astype(np.float32)
        lo = lo.reshape(2 * 128, OUT_COLS)
        off = 0
        for k in range(DEV_LV0, L_MAX + 1):
            cc = 1 << (k - 3)
            unscale = np.float32(2.0 ** (4 * k))
            Tfull[(1 << k) + core:1 << (k + 1):NCORES] = \
                lo[:, off:off + cc].T * unscale
            off += cc
    outv = Tfull[unique.astype(np.int64)]
    return outv, res


def kernel(unique, primitives, identity):
    out, _ = _run(unique, primitives, identity)
    return out


if __name__ == "__main__":
    # tiny smoke run (full shapes) — prefer test.py for the real check
    rng = np.random.default_rng(0)
    u = rng.integers(0, 1 << 17, size=131072).astype(np.int32)
    prims = rng.standard_normal((2, DIM, DIM)).astype(np.float32)
    ones = np.ones((1, DIM), np.float32)
    out = kernel(u, prims, ones)
    print("kernel output", out.shape, out.dtype)
